# revision 40
# baseline (speedup 1.0000x reference)
"""TRN2 Bass kernel for nn_MultiHeadAttention (B=4, S=2048, D=1024, H=16, DH=64).

Sharding (8 cores): core c -> batch b = c//2, head-half hh = c%2 (8 heads each).
Host sums the two half-feature partial outputs per batch.

All-bf16 matmul datapath (bf16 runs 1 cyc/row at ANY moving width on TRN2,
vs fp32r's 4x penalty below 256 cols), rel err ~5e-3. Structure is a
software pipeline over 16 blocks (head h, 1024-wide query block qb), tuned
so the Activation engine (exp, the true floor at ~266us busy) never idles
mid-stream:

  - scores: sT[sk,sq] = kT.T @ qT, two 512-wide bf16 matmuls into a
    [128,1024] PSUM tile; one 1024-wide exp on ACT -> at[st] bf16, kept
    LIVE in a 34-deep SBUF ring for all 16 sk tiles of the block.
  - PV *swapped*: stationary = at[st] 128-col chunk, moving = V_aug[st]
    [sk,65] -> ctx chunk [sq=128, 65] accumulated over st in PSUM. 65
    moving cols/matmul instead of 512 halves PV's PE time; col 64 (ones
    column of V_aug) is the softmax denominator, landing on the PARTITION
    axis of ctx so normalization is reciprocal + one per-partition
    tensor_scalar multiply (no broadcast matmuls).
  - PV+normalize of block n-1 and per-pair ctx transposes (PE transpose
    with identity moving, bf16) interleave into block n's score stream;
    the out-projection interleaves per sq-tile into the last block.
  - v-projection and pairs 1-3 q/k projections run as background work
    pumped into early blocks' ACT-paced slack (pair 0 q/k is the only
    serial head); the 16 v-steps split 8/8 across blocks (0,0)/(0,1),
    with PV(0,0) sliding to the second half of (0,1) so neither early
    block stretches far past its exp budget.
  - x sweeps and weight column loads are single batched DMAs (3-dim DRAM
    access patterns) to keep the SP DMA-issue queue off the critical path.
  - all PSUM-reading element-wise ops run on DVE (GPSIMD cannot access
    PSUM on TRN2); exp feeds PV directly as bf16.

This walrus build accepts only ONE sync-wait per instruction, so after
TileContext scheduling extra waits are split into single-wait NoOps
(legalize_waits).
"""

import sys

if "/opt/trn_rl_repo" not in sys.path:
    sys.path.insert(0, "/opt/trn_rl_repo")

import ml_dtypes
import numpy as np

import concourse.bass as bass
import concourse.mybir as mybir
import concourse.tile as tile
from concourse.bass_utils import run_bass_kernel_spmd

F32 = mybir.dt.float32
BF16 = mybir.dt.bfloat16
EXP = mybir.ActivationFunctionType.Exp

B, S_FULL, D, H = 4, 2048, 1024, 16
DH = 64
NCORES = 8


def legalize_waits(nc, max_waits=1):
    """Split >max_waits sync-waits per instruction into single-wait NoOps on
    the same engine, placed immediately before (per-engine order preserved)."""
    n = 0
    for fn in nc.m.functions:
        for blk in fn.blocks:
            out = []
            for inst in blk.instructions:
                si = inst.sync_info
                if si is not None and len(si.on_wait) > max_waits:
                    waits = list(si.on_wait)
                    for w in waits[:-max_waits]:
                        nop = mybir.InstNoOp(
                            name=f"WSPLIT-{n}", ins=[], outs=[], engine=inst.engine
                        )
                        n += 1
                        nop.sync_info = mybir.SyncInfo(on_wait=[w], on_update=[])
                        out.append(nop)
                    inst.sync_info = mybir.SyncInfo(
                        on_wait=waits[-max_waits:], on_update=list(si.on_update)
                    )
                out.append(inst)
            blk.instructions[:] = out
    return n


def _bcast_ap(src_ap, parts=128):
    """Partition-broadcast a [1, N] AP to [parts, N] via a step-0 dim."""
    return bass.AP(
        tensor=src_ap.tensor,
        offset=src_ap.offset,
        ap=[[0, parts], list(src_ap.ap[-1])],
    )


def build_nc(S=S_FULL, legalize=True):
    NQB = S // 1024  # 1024-wide sq blocks
    NST = S // 128   # sk tiles
    NSB = S // 512   # 512-wide s blocks (projection granularity)
    nc = bass.Bass()
    xT_d = nc.dram_tensor("xt", [D, S], BF16, kind="ExternalInput")
    wq_d = nc.dram_tensor("wq", [128, 4096], BF16, kind="ExternalInput")
    wk_d = nc.dram_tensor("wk", [128, 4096], BF16, kind="ExternalInput")
    wv_d = nc.dram_tensor("wv", [128, 4096], BF16, kind="ExternalInput")
    wo_d = nc.dram_tensor("wo", [128, 4096], BF16, kind="ExternalInput")
    bqk_d = nc.dram_tensor("bqk", [128, 8], F32, kind="ExternalInput")
    bv_d = nc.dram_tensor("bv", [1, 512], F32, kind="ExternalInput")
    bo_d = nc.dram_tensor("bo", [1, 1024], F32, kind="ExternalInput")
    vinit_d = nc.dram_tensor("vinit", [1, NST * 520], BF16, kind="ExternalInput")
    idn_d = nc.dram_tensor("idn", [128, 128], BF16, kind="ExternalInput")
    out_d = nc.dram_tensor("out", [S, 1024], F32, kind="ExternalOutput")

    with tile.TileContext(nc) as tc, nc.allow_low_precision(
        reason="bf16 matmul datapath is intentional; psum accumulation is f32"
    ):
        with tc.tile_pool(name="persist", bufs=1) as pp, \
             tc.tile_pool(name="psP", bufs=2, space="PSUM") as psP:
            qT = pp.tile([128, 4 * S], BF16)
            kT = pp.tile([128, 4 * S], BF16)
            vall = pp.tile([128, NST * 520], BF16)  # per s-tile: 8 heads x 65
            bqk = pp.tile([128, 8], F32)
            bv_b = pp.tile([128, 512], F32)
            bo_b = pp.tile([128, 1024], F32)
            idn = pp.tile([128, 128], BF16)
            ctxN = pp.tile([128, (S // 128) * 512], BF16)  # [sq, (sqTile, feat)]

            # ---- projection sweep 1: pair 0 q/k only; v is pumped as
            # background work into attention block (0,0) ----
            with tc.tile_pool(name="w2p", bufs=1) as w2p:
                # pairs 1-3 weight columns + wv, resident through attention
                wq2 = w2p.tile([128, 3072], BF16)
                wk2 = w2p.tile([128, 3072], BF16)
                wv = w2p.tile([128, 4096], BF16)

                with tc.tile_pool(name="aw", bufs=1) as aw, \
                     tc.tile_pool(name="xp", bufs=2) as xp:
                    wq0 = aw.tile([128, 1024], BF16)
                    wk0 = aw.tile([128, 1024], BF16)

                    def load_x(pool, sb, gen):
                        xt = pool.tile([128, 4096], BF16, tag="x",
                                       name=f"x_{gen}_{sb}")
                        nc.sync.dma_start(
                            out=xt,
                            in_=bass.AP(
                                tensor=xT_d.tensor if hasattr(xT_d, 'tensor') else xT_d,
                                offset=sb * 512,
                                ap=[[S, 128], [128 * S, 8], [1, 512]],
                            ),
                        )
                        return [xt[:, ch * 512:(ch + 1) * 512] for ch in range(8)]

                    def qk_group(wmat, wcol0, dstT, bcol, p, sb, xs):
                        stride = wmat.shape[1] // 8
                        ps_q = psP.tile([128, 512], F32, tag="pp", name="ps_q")
                        for ch in range(8):
                            nc.tensor.matmul(
                                ps_q,
                                wmat[:, wcol0 + ch * stride:
                                     wcol0 + ch * stride + 128],
                                xs[ch],
                                start=(ch == 0),
                                stop=(ch == 7),
                            )
                        nc.vector.tensor_scalar_add(
                            dstT[:, p * S + sb * 512: p * S + (sb + 1) * 512],
                            ps_q,
                            bqk[:, bcol + p: bcol + p + 1],
                        )

                    # DMA issue order: first-needed bytes first
                    xs0 = load_x(xp, 0, 1)
                    for w_d, wdst in ((wq_d, wq0), (wk_d, wk0)):  # pair-0 cols
                        nc.sync.dma_start(
                            out=wdst,
                            in_=bass.AP(tensor=w_d, offset=0,
                                        ap=[[4096, 128], [512, 8], [1, 128]]))
                    nc.sync.dma_start(out=bqk, in_=bqk_d[:, :])
                    nc.sync.dma_start(out=idn, in_=idn_d[:, :])
                    nc.sync.dma_start(out=bv_b, in_=_bcast_ap(bv_d[:, :]))
                    # V_aug template (1.0 in each head's 65th col)
                    nc.sync.dma_start(out=vall, in_=_bcast_ap(vinit_d[:, :]))
                    nc.sync.dma_start(out=bo_b, in_=_bcast_ap(bo_d[:, :]))

                    for sb in range(NSB):
                        xs = xs0 if sb == 0 else load_x(xp, sb, 1)
                        qk_group(wq0, 0, qT, 0, 0, sb, xs)
                        qk_group(wk0, 0, kT, 4, 0, sb, xs)

                nc.sync.dma_start(out=wv, in_=wv_d[:, :])  # needed block (0,0)
                for w_d, wdst in ((wq_d, wq2), (wk_d, wk2)):  # pairs 1-3 cols
                    nc.sync.dma_start(
                        out=wdst,
                        in_=bass.AP(tensor=w_d, offset=128,
                                    ap=[[4096, 128], [512, 8], [1, 384]]))

                # ---- attention pipeline ----
                # Block n = (h, qb): emit scores+exp for block n interleaved
                # with PV+normalize(+pair transposes) of block n-1 and, late
                # in the schedule, the per-sq-tile out-projection.
                with tc.tile_pool(name="p2", bufs=2) as p2, \
                     tc.tile_pool(name="oc", bufs=1) as oc, \
                     tc.tile_pool(name="at", bufs=34) as atp, \
                     tc.tile_pool(name="sm", bufs=4) as sm, \
                     tc.tile_pool(name="cot", bufs=2) as cot, \
                     tc.tile_pool(name="psS", bufs=2, space="PSUM") as psS, \
                     tc.tile_pool(name="psC", bufs=2, space="PSUM") as psC:
                    wo = oc.tile([128, 4096], BF16)
                    ctxT = oc.tile([128, 4 * S], BF16)  # [feat, (fc, sq)]
                    nc.sync.dma_start(out=wo, in_=wo_d[:, :])

                    def bg_gen():
                        # v projection (16 steps), consumed by PV from block
                        # (0,1) onwards
                        for sb in range(NSB):
                            xs = load_x(p2, sb, 9)
                            for t4 in range(4):
                                st = sb * 4 + t4
                                ps_v = psP.tile([128, 512], F32, tag="pp",
                                                name=f"ps_v{st}")
                                for ch in range(8):
                                    nc.tensor.matmul(
                                        ps_v,
                                        xs[ch][:, t4 * 128:(t4 + 1) * 128],
                                        wv[:, ch * 512:(ch + 1) * 512],
                                        start=(ch == 0),
                                        stop=(ch == 7),
                                    )
                                dst = vall[:, st * 520:(st + 1) * 520].rearrange(
                                    "p (h e) -> p h e", e=65
                                )[:, :, 0:64]
                                nc.vector.tensor_add(
                                    dst,
                                    ps_v.rearrange("p (h e) -> p h e", e=64),
                                    bv_b.rearrange("p (h e) -> p h e", e=64),
                                )
                                yield
                        # pairs 1-3 q/k projections
                        for p in (1, 2, 3):
                            for sb in range(NSB):
                                xs = load_x(p2, sb, 1 + p)
                                qk_group(wq2, (p - 1) * 128, qT, 0, p, sb, xs)
                                yield
                                qk_group(wk2, (p - 1) * 128, kT, 4, p, sb, xs)
                                yield

                    pass2 = bg_gen()

                    def pv_gen(h, qb, ats):
                        """PV + normalize (+ pair transpose when h is odd) for
                        one block, one sq-chunk per yield (8 yields)."""
                        for sqc in range(8):
                            t = qb * 8 + sqc
                            ps_c = psC.tile([128, 512], F32, tag="pc")
                            for st in range(NST):
                                nc.tensor.matmul(
                                    ps_c[:, 0:65],
                                    ats[st][:, sqc * 128:(sqc + 1) * 128],
                                    vall[:, st * 520 + h * 65:
                                         st * 520 + (h + 1) * 65],
                                    start=(st == 0),
                                    stop=(st == NST - 1),
                                )
                            rsum = sm.tile([128, 1], F32, tag="rsum")
                            nc.vector.reciprocal(rsum, ps_c[:, 64:65])
                            nc.vector.tensor_scalar_mul(
                                ctxN[:, t * 512 + h * 64: t * 512 + (h + 1) * 64],
                                ps_c[:, 0:64],
                                rsum,
                            )
                            if h % 2 == 1:
                                p = h // 2
                                tpool = psC if h == 7 else psP
                                ps_tp = tpool.tile([128, 512], F32,
                                                   tag="pc" if h == 7 else "pp",
                                                   name=f"ps_tp{h}_{t}")
                                tp = ps_tp[:, 0:64].bitcast(BF16)
                                nc.tensor.matmul(
                                    tp,
                                    ctxN[:, t * 512 + p * 128:
                                         t * 512 + (p + 1) * 128],
                                    idn,
                                    is_transpose=True,
                                    start=True,
                                    stop=True,
                                )
                                nc.vector.tensor_copy(
                                    ctxT[:, p * S + t * 128: p * S + (t + 1) * 128],
                                    tp,
                                )
                            yield

                    def oproj_gen():
                        """Out-projection, one sq-tile per yield (16)."""
                        for t in range(NST):
                            for half in range(2):
                                ps_o = psP.tile([128, 512], F32, tag="pp",
                                                name=f"ps_o{t}_{half}")
                                for fc in range(4):
                                    nc.tensor.matmul(
                                        ps_o,
                                        ctxT[:, fc * S + t * 128:
                                             fc * S + (t + 1) * 128],
                                        wo[:, fc * 1024 + half * 512:
                                           fc * 1024 + (half + 1) * 512],
                                        start=(fc == 0),
                                        stop=(fc == 3),
                                    )
                                ot = cot.tile([128, 512], F32, tag="ot")
                                nc.vector.tensor_add(
                                    ot, ps_o, bo_b[:, half * 512:(half + 1) * 512]
                                )
                                nc.sync.dma_start(
                                    out=out_d[t * 128:(t + 1) * 128,
                                              half * 512:(half + 1) * 512],
                                    in_=ot,
                                )
                            yield

                    oproj = oproj_gen()
                    pv_prev = None
                    for h in range(8):
                        p = h // 2
                        r0 = 64 * (h % 2)
                        for qb in range(NQB):
                            ats = []
                            for st in range(NST):
                                if (h == 0 and qb == 0 and st % 2 == 0) or \
                                   (h == 0 and qb == 1 and st < 8) or \
                                   (h == 1 and st % 4 == 3) or \
                                   (2 <= h < 6 and st in (6, NST - 2)):
                                    next(pass2, None)
                                ps_s = psS.tile([128, 1024], F32, tag="ps")
                                for half in range(2):
                                    nc.tensor.matmul(
                                        ps_s[:, half * 512:(half + 1) * 512],
                                        kT[r0:r0 + 64,
                                           p * S + st * 128: p * S + (st + 1) * 128],
                                        qT[r0:r0 + 64,
                                           p * S + qb * 1024 + half * 512:
                                           p * S + qb * 1024 + (half + 1) * 512],
                                        start=True,
                                        stop=True,
                                    )
                                at = atp.tile([128, 1024], BF16, tag="at")
                                nc.scalar.activation(at, ps_s, EXP, scale=0.125)
                                ats.append(at)
                                if pv_prev is not None:
                                    if h == 0 and qb == 1:
                                        if st >= 8:
                                            next(pv_prev, None)
                                    elif st % 2 == 1:
                                        next(pv_prev, None)
                                if h == 7 and qb == 1 and st % 2 == 0 and st >= 2:
                                    # tiles 0-7 (qb=0) became complete during
                                    # the previous block's PV
                                    next(oproj, None)
                            pv_prev = pv_gen(h, qb, ats)
                    for _ in pv_prev:
                        next(oproj, None)
                    for _ in oproj:
                        pass

    if legalize:
        legalize_waits(nc)
    return nc


def pack_core_inputs(c, x, Wq, bq, Wk, bk, Wv, bv, Wo, bo, S=S_FULL):
    """Pack full-model inputs into core c's device tensors."""
    b = c // 2
    hh = c % 2
    hs = slice(hh * 8, hh * 8 + 8)

    def bf(a):
        return np.ascontiguousarray(a.astype(ml_dtypes.bfloat16))

    def pack_w(W):  # [8, D, DH] -> [128, 4096]: free = chunk*512 + (h*64+dh)
        W2 = np.transpose(W, (1, 0, 2)).reshape(D, 512)      # [d, h*dh]
        return np.ascontiguousarray(
            np.transpose(W2.reshape(8, 128, 512), (1, 0, 2)).reshape(128, 4096)
        )

    xT = np.ascontiguousarray(x[b].T)                         # [D, S]
    wq = pack_w(Wq[hs])
    wk = pack_w(Wk[hs])
    wv = pack_w(Wv[hs])
    # Wo rows for this half's features: [512, 1024] -> [128, 4*1024]
    Wr = Wo[hh * 512:(hh + 1) * 512]
    wo = np.ascontiguousarray(
        np.transpose(Wr.reshape(4, 128, 1024), (1, 0, 2)).reshape(128, 4096)
    )
    bqk = np.concatenate(
        [bq[hs].reshape(4, 128).T, bk[hs].reshape(4, 128).T], axis=1
    )                                                         # [128, 8]
    bvp = bv[hs].reshape(1, 512)
    bop = (0.5 * bo).reshape(1, 1024)
    NST = S // 128
    vinit = np.zeros((1, NST * 520), dtype=np.float32)
    vinit[0, 64::65] = 1.0
    return {
        "vinit": bf(vinit),
        "idn": bf(np.eye(128, dtype=np.float32)),
        "xt": bf(xT),
        "wq": bf(wq),
        "wk": bf(wk),
        "wv": bf(wv),
        "wo": bf(wo),
        "bqk": np.ascontiguousarray(bqk).astype(np.float32),
        "bv": bvp.astype(np.float32),
        "bo": bop.astype(np.float32),
    }


_NC_CACHE = {}


def _get_nc(S=S_FULL):
    if S not in _NC_CACHE:
        _NC_CACHE[S] = build_nc(S)
    return _NC_CACHE[S]


def kernel(x, Wq, bq, Wk, bk, Wv, bv, Wo, bo, _trace=False):
    x, Wq, bq, Wk, bk, Wv, bv, Wo, bo = (
        np.asarray(a, dtype=np.float32) for a in (x, Wq, bq, Wk, bk, Wv, bv, Wo, bo)
    )
    nc = _get_nc()
    in_maps = [
        pack_core_inputs(c, x, Wq, bq, Wk, bk, Wv, bv, Wo, bo) for c in range(NCORES)
    ]
    res = run_bass_kernel_spmd(nc, in_maps, list(range(NCORES)), trace=_trace)
    out = np.empty((B, S_FULL, D), dtype=np.float32)
    for b in range(B):
        out[b] = res.results[2 * b]["out"] + res.results[2 * b + 1]["out"]
    if _trace:
        kernel.last_results = res
    return out
